# revision 1
# baseline (speedup 1.0000x reference)
"""CTC loss (nn_CTCCriterion) Trainium2 Bass kernel.

Strategy: pure data parallelism, 4 sequences per core x 8 cores.
The CTC forward DP is run in the exp (probability) domain so each step is
  P[t] <- (P[t] + P[t-1]) * qs[i,t],   qs = (clamped x / rowsum) * e^BETA
with a deterministic per-step prescale e^BETA to keep f32 in range, an
adaptive renormalization every RESCALE_K steps, and the skip-penalty
injection P[-1] = exp((BETA-5)*i) for the first few steps (beyond ~i=48
its contribution is below f32 resolution of the result).
The one-hot gather lmatch = x @ onehot and the row sums are PE matmuls.
"""

import numpy as np

S, N, C, L = 1024, 32, 128, 128
T = 2 * L + 1  # 257
NSEQ = 4       # sequences per core
NCORES = 8
BETA = 4.17
RESCALE_K = 64
INJ_STOP = 48  # stop skip-penalty injection after this step
CH = 32        # q streaming chunk (steps per DMA)

_CACHE = {}


def _build():
    import concourse.bacc as bacc
    import concourse.mybir as mybir
    from concourse.tile import TileContext

    f32 = mybir.dt.float32
    Alu = mybir.AluOpType
    Act = mybir.ActivationFunctionType

    nc = bacc.Bacc("TRN2")
    xt = nc.dram_tensor("xt", [C, NSEQ * S], f32, kind="ExternalInput")
    oh = nc.dram_tensor("oh", [C, NSEQ * T], f32, kind="ExternalInput")
    qdram = nc.dram_tensor("qtmp", [NSEQ, S * T], f32, kind="Internal")
    loss = nc.dram_tensor("loss", [NSEQ, 1], f32, kind="ExternalOutput")

    with TileContext(nc) as tc:
        from contextlib import ExitStack
        with ExitStack() as ctx:
            singles = ctx.enter_context(tc.tile_pool(name="singles", bufs=1))
            ppool = ctx.enter_context(tc.tile_pool(name="psum", bufs=4, space="PSUM"))
            spool = ctx.enter_context(tc.tile_pool(name="scal", bufs=4))
            stpool = ctx.enter_context(tc.tile_pool(name="stage", bufs=3))
            qpool = ctx.enter_context(tc.tile_pool(name="qstream", bufs=3))

            # ---- load inputs ----
            xt_sb = singles.tile([C, NSEQ * S], f32)
            oh_sb = singles.tile([C, NSEQ * T], f32)
            nc.sync.dma_start(xt_sb[:], xt[:, :])
            nc.sync.dma_start(oh_sb[:], oh[:, :])
            ones = singles.tile([C, 1], f32)
            nc.any.memset(ones[:], float(np.exp(-BETA)))

            # clamp to 1e-5 (in place)
            nc.vector.tensor_scalar_max(xt_sb[:], xt_sb[:], 1e-5)
            tc.strict_bb_all_engine_barrier()

            # ---- phase 1: q = (x/rowsum)*e^BETA  -> qdram ----
            for s in range(NSEQ):
                for it in range(S // C):  # 8 i-tiles of 128
                    lhsT = xt_sb[:, s * S + it * C : s * S + (it + 1) * C]
                    pg = ppool.tile([C, T], f32, tag="pg")
                    ps = ppool.tile([C, 1], f32, tag="ps")
                    nc.tensor.matmul(pg[:], lhsT, oh_sb[:, s * T : (s + 1) * T])
                    nc.tensor.matmul(ps[:], lhsT, ones[:])
                    rec = spool.tile([C, 1], f32, tag="rec")
                    nc.vector.reciprocal(rec[:], ps[:])
                    st = stpool.tile([C, T], f32, tag="st")
                    nc.vector.tensor_scalar_mul(st[:], pg[:], rec[:])
                    nc.sync.dma_start(
                        qdram[s : s + 1, it * C * T : (it + 1) * C * T].rearrange(
                            "s (i t) -> (s i) t", t=T
                        ),
                        st[:],
                    )

            tc.strict_bb_all_engine_barrier()

            # ---- phase 2: DP over S steps ----
            # P buffer: col j holds state t=j-1 (col 0 = guard/injection)
            P = singles.tile([NSEQ, T + 1], f32)
            U = singles.tile([NSEQ, T + 1], f32)
            A = singles.tile([NSEQ, 1], f32)
            Mx = spool.tile([NSEQ, 1], f32, tag="mx")
            Rv = spool.tile([NSEQ, 1], f32, tag="rv")
            Lg = spool.tile([NSEQ, 1], f32, tag="lg")

            nc.vector.memset(A[:], 0.0)
            # init P: P[:, j] = exp(-5*(j-1)) for j>=1 (col 0 = guard, set per step)
            pinit = np.zeros((NSEQ, T + 1), np.float32)
            pinit[:, 1:] = np.exp(-5.0 * np.arange(T, dtype=np.float64)).astype(
                np.float32
            )
            pinit[:, 0] = 1.0
            pinit_dram = nc.inline_tensor(pinit, name="pinit")
            nc.sync.dma_start(P[:], pinit_dram[:, :])

            for c in range(S // CH):
                qs = qpool.tile([NSEQ, CH * T], f32, tag="qs")
                nc.sync.dma_start(
                    qs[:], qdram[0:NSEQ, c * CH * T : (c + 1) * CH * T]
                )
                for j in range(CH):
                    i = c * CH + j
                    # guard/injection column (vector engine: keep DP chain on DVE)
                    if i < INJ_STOP:
                        nc.vector.memset(P[:, 0:1], float(np.exp((BETA - 5.0) * i)))
                    elif i == INJ_STOP:
                        nc.vector.memset(P[:, 0:1], 0.0)
                    # U[t] = P[t] + P[t-1]
                    nc.vector.tensor_tensor(
                        U[:, 1 : T + 1], P[:, 1 : T + 1], P[:, 0:T], Alu.add
                    )
                    # P[t] = U[t] * q[i, t]
                    nc.vector.tensor_tensor(
                        P[:, 1 : T + 1],
                        U[:, 1 : T + 1],
                        qs[:, j * T : (j + 1) * T],
                        Alu.mult,
                    )
                    if (i + 1) % RESCALE_K == 0:
                        nc.vector.tensor_reduce(
                            Mx[:], P[:, 1 : T + 1], mybir.AxisListType.X, Alu.max
                        )
                        nc.vector.reciprocal(Rv[:], Mx[:])
                        nc.vector.tensor_scalar_mul(
                            P[:, 1 : T + 1], P[:, 1 : T + 1], Rv[:]
                        )
                        nc.scalar.activation(Lg[:], Mx[:], Act.Ln)
                        nc.vector.tensor_tensor(A[:], A[:], Lg[:], Alu.add)

            # ---- phase 3: loss = S*BETA - (log(P[T-1]+P[T-2]) + A) ----
            fin = spool.tile([NSEQ, 1], f32, tag="fin")
            nc.vector.tensor_tensor(fin[:], P[:, T : T + 1], P[:, T - 1 : T], Alu.add)
            nc.scalar.activation(fin[:], fin[:], Act.Ln)
            nc.vector.tensor_tensor(fin[:], fin[:], A[:], Alu.add)
            lout = spool.tile([NSEQ, 1], f32, tag="lout")
            nc.scalar.activation(lout[:], fin[:], Act.Copy, bias=float(S * BETA), scale=-1.0)
            nc.sync.dma_start(loss[:, :], lout[:])

    nc.compile()
    return nc


def kernel(input, targets):
    from concourse.bass_utils import run_bass_kernel_spmd

    if "nc" not in _CACHE:
        _CACHE["nc"] = _build()
    nc = _CACHE["nc"]

    x = np.asarray(input, np.float32)
    tg = np.asarray(targets)
    in_maps = []
    for cid in range(NCORES):
        sl = x[:, NSEQ * cid : NSEQ * (cid + 1), :]          # (S, 4, C)
        xtc = np.ascontiguousarray(sl.transpose(2, 1, 0)).reshape(C, NSEQ * S)
        lab = np.zeros((NSEQ, T), np.int64)
        lab[:, 1::2] = tg[:, NSEQ * cid : NSEQ * (cid + 1)].T
        ohc = (np.arange(C)[:, None, None] == lab[None, :, :]).astype(np.float32)
        in_maps.append({"xt": xtc, "oh": ohc.reshape(C, NSEQ * T)})

    import os

    kwargs = {}
    if os.environ.get("CTC_TRACE"):
        kwargs = {"trace": True}
    res = run_bass_kernel_spmd(nc, in_maps, core_ids=list(range(NCORES)), **kwargs)
    if os.environ.get("CTC_TRACE"):
        _CACHE["exec_time_ns"] = res.exec_time_ns
        _CACHE["trace"] = res.instructions_and_trace
    total = 0.0
    for cid in range(NCORES):
        total += float(np.sum(res.results[cid]["loss"].astype(np.float64)))
    return np.float32(total / N)

